# revision 1
# baseline (speedup 1.0000x reference)
"""Masked multi-head attention kernel for Trainium2 (Bass/Tile), 8-core SPMD.

Problem: BH=64 heads of S=2048, D=64 attention with a dense bool mask,
scale = 1/sqrt(1024).  Sharded 8 heads per NeuronCore (no cross-core comm).

Per-core dataflow (heads processed in pairs):
  - Q,K loaded f32, cast to bf16, PE-transposed into QT/KT slabs [d, S]
    with head A on partitions 0-63 and head B on partitions 64-127.
  - S^T[k,q] = K @ Q^T computed with row-tiled paired matmuls (head A in
    PE rows 0-63, head B in rows 64-127; they run concurrently).
  - The bool mask is applied on the PE: mask[q,k] tiles (DMA-cast u8->bf16)
    are used as the stationary operand against a -960*I identity, which
    accumulates -960*mask^T into the same PSUM tile.  After the ACT exp
    with scale=1/32 this is exp(S - 30*mask) ~= 0 for masked entries.
  - exp on the scalar engine PSUM->SBUF (bf16 out) builds the P^T slab.
  - AV: for each k-chunk, stationary [V | 1] (M=65) streams P^T, giving
    O^T (rows 0-63) and the softmax denominators l (row 64) in PSUM.
  - Epilogue: PE-transpose O^T back to natural [q, d], reciprocal of l,
    per-partition scale on the vector engine, natural DMA store.
"""

import os
import sys

sys.path.insert(0, "/opt/trn_rl_repo")

import numpy as np

import concourse.bass as bass
import concourse.mybir as mybir
import concourse.tile as tile
from concourse import bacc
from concourse.bass_utils import run_bass_kernel_spmd
from concourse.masks import make_identity

N_CORES = 8
BH, S_FULL, D = 64, 2048, 64
H_PER_CORE = BH // N_CORES  # 8
P = 128  # SBUF/PSUM partitions
KCH = 128  # k-chunk (S^T partition tile)
SCALE = 1.0 / 32.0  # 1/sqrt(1024) per the module spec
NEGC = -960.0  # -960/32 = -30 after the ACT scale -> exp ~ 9e-14


def build_attention(tc, o_ap, q_ap, k_ap, v_ap, m_ap, H, S, qch,
                    do_mask=True, do_exp=True, do_av=True, do_maskdma=None,
                    extra_exp=False, extra_mask=False, extra_qk=False,
                    extra_av=False):
    if do_maskdma is None:
        do_maskdma = do_mask
    nc = tc.nc
    dt = mybir.dt
    n_pairs = H // 2
    n_kch = S // KCH
    n_qt = S // P
    n_qch = S // qch
    QS = qch // P  # q-subtiles per chunk

    with (
        tc.tile_pool(name="const", bufs=1) as constp,
        tc.tile_pool(name="stage", bufs=8) as stagep,
        tc.tile_pool(name="qkslab", bufs=2) as qkp,
        tc.tile_pool(name="vp", bufs=4 * n_kch) as vpool,
        tc.tile_pool(name="maskp", bufs=4) as maskp,
        tc.tile_pool(name="ptp", bufs=16) as ptp,
        tc.tile_pool(name="op", bufs=4) as opool,
        tc.tile_pool(name="smallp", bufs=8) as smallp,
        tc.tile_pool(name="ps_s", bufs=3, space="PSUM") as ps_s,
        tc.tile_pool(name="ps_o", bufs=2, space="PSUM") as ps_o,
    ):
        identB = constp.tile([P, P], dt.bfloat16)
        make_identity(nc, identB)
        identF = constp.tile([P, P], dt.float32)
        make_identity(nc, identF)
        negI = constp.tile([P, P], dt.bfloat16)
        nc.gpsimd.memset(negI, 0.0)
        nc.gpsimd.affine_select(
            out=negI,
            in_=negI,
            compare_op=mybir.AluOpType.not_equal,
            fill=NEGC,
            base=0,
            pattern=[[-1, P]],
            channel_multiplier=1,
        )

        for pr in range(n_pairs):
            heads = (2 * pr, 2 * pr + 1)

            # ---- Q/K: load f32, cast bf16, xbar-DMA-transpose into [d2, S] ----
            QT2 = qkp.tile([P, S], dt.bfloat16, tag="qt2")
            KT2 = qkp.tile([P, S], dt.bfloat16, tag="kt2")
            for src_ap, slab in ((q_ap, QT2), (k_ap, KT2)):
                for t in range(n_qt):
                    qn = stagep.tile([P, P], dt.bfloat16, tag="qn")
                    for hi, h in enumerate(heads):
                        stf = stagep.tile([P, D], dt.float32, tag="ldstage")
                        nc.sync.dma_start(stf[:], src_ap[h, t * P : (t + 1) * P, :])
                        nc.vector.tensor_copy(qn[:, hi * D : (hi + 1) * D], stf[:])
                    pst = ps_o.tile([P, P], dt.bfloat16, tag="po")
                    nc.tensor.transpose(pst[:], qn[:], identB[:])
                    nc.vector.tensor_copy(slab[:, t * P : (t + 1) * P], pst[:])

            # ---- V: load f32, cast bf16 into [128, 65] tiles with ones col ----
            v2 = [[None] * n_kch for _ in range(2)]
            for hi, h in enumerate(heads):
                for ki in range(n_kch):
                    stf = stagep.tile([P, D], dt.float32, tag="ldstage")
                    nc.sync.dma_start(stf[:], v_ap[h, ki * P : (ki + 1) * P, :])
                    t2 = vpool.tile([P, D + 1], dt.bfloat16, tag="v2")
                    nc.vector.tensor_copy(t2[:, 0:D], stf[:])
                    nc.vector.memset(t2[:, D : D + 1], 1.0)
                    v2[hi][ki] = t2

            # O^T accumulators across k-quarters: [65, qch] f32 per (head, qc)
            osb_acc = [
                [
                    opool.tile(
                        [D + 1, qch],
                        dt.float32,
                        tag="os",
                        name=f"os{pr}_{hi}_{qc}",
                        bufs=4 * n_qch,
                    )
                    for qc in range(n_qch)
                ]
                for hi in range(2)
            ]

            NQ = min(4, n_kch)  # k-chunks per mask slab
            n_quart = n_kch // NQ
            for qt_ in range(n_quart):
                # mask^T quarter tiles: m_ap holds the HOST-TRANSPOSED mask
                # [H, S_k, S_q]; tile covers NQ k-chunks x all q (2KB runs):
                # mt[p, kl*S + j] = maskT[h, (qt_*NQ + kl)*128 + p, j]
                mts = [None, None]
                if do_maskdma:
                    for hi, h in enumerate(heads):
                        mt = maskp.tile([P, NQ * S], dt.bfloat16, tag="mask")
                        src = m_ap[h, qt_ * NQ * P : (qt_ + 1) * NQ * P, :].rearrange(
                            "(kl p) j -> p kl j", p=P
                        )
                        dst = mt[:].rearrange("p (kl j) -> p kl j", kl=NQ)
                        nc.gpsimd.dma_start(dst, src)
                        mts[hi] = mt
                elif do_mask:
                    dummy = maskp.tile([P, NQ * S], dt.bfloat16, tag="mask")
                    nc.vector.memset(dummy[:], 0.0)
                    mts = [dummy, dummy]

                for qc in range(n_qch):
                    q0 = qc * qch
                    # S^T = K Q^T (paired row-tiled) minus C*mask^T, then exp.
                    # k-chunks processed in pairs sharing one [128, 2*qch]
                    # PSUM tile so the exp runs at FD=2*qch.
                    NP2 = NQ // 2
                    pts = [[None] * NP2 for _ in range(2)]
                    for kp in range(NP2):
                        for hi in range(2):
                            st_ = ps_s.tile([P, 2 * qch], dt.float32, tag="st")
                            for half in range(2):
                                ki = qt_ * NQ + 2 * kp + half
                                k0 = ki * KCH
                                reg = st_[:, half * qch : (half + 1) * qch]
                                nc.tensor.matmul(
                                    reg,
                                    KT2[hi * D : (hi + 1) * D, k0 : k0 + KCH],
                                    QT2[hi * D : (hi + 1) * D, q0 : q0 + qch],
                                    start=True,
                                    stop=not do_mask,
                                )
                                if do_mask:
                                    kl = 2 * kp + half
                                    if extra_mask:
                                        nc.tensor.matmul(
                                            reg,
                                            negI[:],
                                            mts[hi][:, kl * S + q0 : kl * S + q0 + qch],
                                            start=False,
                                            stop=False,
                                        )
                                    if extra_qk:
                                        ki2 = qt_ * NQ + 2 * kp + half
                                        nc.tensor.matmul(
                                            reg,
                                            KT2[hi * D : (hi + 1) * D, ki2 * KCH : ki2 * KCH + KCH],
                                            QT2[hi * D : (hi + 1) * D, q0 : q0 + qch],
                                            start=False,
                                            stop=False,
                                        )
                                    nc.tensor.matmul(
                                        reg,
                                        negI[:],
                                        mts[hi][:, kl * S + q0 : kl * S + q0 + qch],
                                        start=False,
                                        stop=True,
                                    )
                            pt = ptp.tile([P, 2 * qch], dt.bfloat16, tag="pt")
                            if do_exp:
                                if extra_exp:
                                    ptx = ptp.tile([P, 2 * qch], dt.bfloat16, tag="ptx", bufs=4)
                                    nc.scalar.activation(
                                        ptx[:], st_[:],
                                        mybir.ActivationFunctionType.Exp, scale=SCALE,
                                    )
                                nc.scalar.activation(
                                    pt[:],
                                    st_[:],
                                    mybir.ActivationFunctionType.Exp,
                                    scale=SCALE,
                                )
                            else:
                                nc.vector.tensor_copy(pt[:], st_[:])
                            pts[hi][kp] = pt

                    if not do_av:
                        for hi, h in enumerate(heads):
                            of = opool.tile([P, D], dt.float32, tag="of")
                            nc.vector.tensor_copy(of[:], pts[hi][0][:, 0:D])
                            nc.sync.dma_start(o_ap[h, q0 : q0 + P, :], of[:])
                        continue

                    # partial O^T = [V | 1]^T @ P^T over this quarter's k
                    for hi in range(2):
                        po = ps_o.tile([D + 1, qch], dt.float32, tag="po")
                        for kl in range(NQ):
                            if extra_av:
                                nc.tensor.matmul(
                                    po[:],
                                    v2[hi][qt_ * NQ + kl][:],
                                    pts[hi][kl // 2][:, (kl % 2) * qch : (kl % 2 + 1) * qch],
                                    start=False if kl > 0 else (qt_ == 0 and False),
                                    stop=False,
                                    skip_group_check=True,
                                )
                            nc.tensor.matmul(
                                po[:],
                                v2[hi][qt_ * NQ + kl][:],
                                pts[hi][kl // 2][:, (kl % 2) * qch : (kl % 2 + 1) * qch],
                                start=(kl == 0),
                                stop=(kl == NQ - 1),
                                skip_group_check=True,
                            )
                        if qt_ == 0:
                            nc.vector.tensor_copy(osb_acc[hi][qc][:], po[:])
                        else:
                            nc.vector.tensor_add(
                                osb_acc[hi][qc][:], osb_acc[hi][qc][:], po[:]
                            )

            if not do_av:
                continue

            # epilogue: transpose O^T -> O, normalize by l, store
            for hi, h in enumerate(heads):
                for qc in range(n_qch):
                    q0 = qc * qch
                    osb = osb_acc[hi][qc]
                    for ot in range(QS):
                        pst2 = ps_o.tile([P, D + 1], dt.float32, tag="po")
                        nc.tensor.transpose(
                            pst2[:],
                            osb[:, ot * P : (ot + 1) * P],
                            identF[0 : D + 1, 0 : D + 1],
                        )
                        rc = smallp.tile([P, 1], dt.float32, tag="rc")
                        nc.vector.reciprocal(rc[:], pst2[:, D : D + 1])
                        of = opool.tile([P, D], dt.float32, tag="of")
                        nc.vector.tensor_scalar_mul(of[:], pst2[:, 0:D], rc[:])
                        nc.sync.dma_start(
                            o_ap[h, q0 + ot * P : q0 + (ot + 1) * P, :], of[:]
                        )


def build_program(H=H_PER_CORE, S=S_FULL, qch=512, repeat=1, **flags):
    nc = bacc.Bacc()
    q = nc.dram_tensor("q", [H, S, D], mybir.dt.float32, kind="ExternalInput")
    k = nc.dram_tensor("k", [H, S, D], mybir.dt.float32, kind="ExternalInput")
    v = nc.dram_tensor("v", [H, S, D], mybir.dt.float32, kind="ExternalInput")
    m = nc.dram_tensor("m", [H, S, S], mybir.dt.uint8, kind="ExternalInput")
    o = nc.dram_tensor("o", [H, S, D], mybir.dt.float32, kind="ExternalOutput")
    with tile.TileContext(nc) as tc:
        for _ in range(repeat):
            build_attention(
                tc, o.ap(), q.ap(), k.ap(), v.ap(), m.ap(), H=H, S=S, qch=qch, **flags
            )
    nc.compile()
    return nc


_CACHE = {}
LAST_RESULTS = None


def kernel(queries, keys, values, mask):
    global LAST_RESULTS
    if "nc" not in _CACHE:
        _CACHE["nc"] = build_program()
    nc = _CACHE["nc"]

    queries = np.ascontiguousarray(queries, dtype=np.float32)
    keys = np.ascontiguousarray(keys, dtype=np.float32)
    values = np.ascontiguousarray(values, dtype=np.float32)
    # ship the mask transposed ([BH, k, q]) so on-device tiles are k-major
    mask_u8 = np.ascontiguousarray(np.asarray(mask).transpose(0, 2, 1)).view(np.uint8)

    in_maps = []
    for c in range(N_CORES):
        sl = slice(c * H_PER_CORE, (c + 1) * H_PER_CORE)
        in_maps.append(
            {
                "q": queries[sl],
                "k": keys[sl],
                "v": values[sl],
                "m": mask_u8[sl],
            }
        )

    trace = bool(int(os.environ.get("ATTN_TRACE", "0")))
    res = run_bass_kernel_spmd(
        nc, in_maps, core_ids=list(range(N_CORES)), trace=trace
    )
    LAST_RESULTS = res
    return np.concatenate([r["o"] for r in res.results], axis=0)



# revision 3
# speedup vs baseline: 1.7730x; 1.7730x over previous
"""Masked multi-head attention kernel for Trainium2 (Bass/Tile), 8-core SPMD.

Problem: BH=64 heads of S=2048, D=64 attention with a dense bool mask,
scale = 1/sqrt(1024).  Sharded 8 heads per NeuronCore (no cross-core comm).

Per-core dataflow (heads processed in pairs; ACT-exp is the design bottleneck):
  - Q,K: SWDGE cast-DMA f32->bf16 HBM->HBM into a pair-interleaved scratch
    [S, 128] (head A in cols 0:64, head B in 64:128), then ONE HWDGE xbar
    transpose-DMA per (pair, tensor) builds the [d2, S] slab directly in SBUF.
    No PE transposes, no DVE evacuation for the slabs.
  - V: SWDGE cast-DMA into [128, 16*65] chunk-major tiles; col 64 of each
    65-wide group memset to 1.0 once (gives softmax denominators in AV).
  - mask: host sends KEEP mask (1=keep) transposed [H, S_k, S_q] u8; SWDGE
    cast-DMA u8->bf16 into per-quarter slabs laid out [p, qc, kl, j] so the
    multiply operand for each (qc, k-pair) is one contiguous [128, 1024].
  - S^T = K Q^T row-paired on the PE (head A rows 0:64, head B rows 64:128,
    alternating -> LDWEIGHTS hides).  exp on ACT (scale=1/32) PSUM->SBUF bf16.
  - mask applied POST-exp as one DVE tensor_mul [128,1024] (bf16 2x mode);
    exact zeros for masked entries.
  - AV: stationary [V|1] (M=65) streams masked P^T, accumulating over the 8
    k-chunks of a half directly in PSUM; halves combined with one DVE add.
  - Epilogue per (qc, head): 4 PE transposes into one packed PSUM tile
    [128, 4*65], one strided reciprocal, 4 per-partition scales, one store.
"""

import os
import sys

sys.path.insert(0, "/opt/trn_rl_repo")

import numpy as np

import concourse.bass as bass
import concourse.mybir as mybir
import concourse.tile as tile
from concourse import bacc
from concourse.bass_utils import run_bass_kernel_spmd
from concourse.masks import make_identity

N_CORES = 8
BH, S_FULL, D = 64, 2048, 64
H_PER_CORE = BH // N_CORES  # 8
P = 128  # SBUF/PSUM partitions
KCH = 128  # k-chunk rows
QCH = 512  # q-chunk cols
SCALE = 1.0 / 32.0  # 1/sqrt(1024) per the module spec


def build_attention(tc, o_ap, q_ap, k_ap, v_ap, m_ap, H, S):
    nc = tc.nc
    dt = mybir.dt
    n_pairs = H // 2
    n_kch = S // KCH  # 16 k-chunks per head
    n_qc = S // QCH  # 4 q-chunks
    NKH = n_kch // 2  # 8 k-chunks per half
    n_quart = 4  # mask DMA granularity: quarter of the k range
    KLQ = n_kch // n_quart  # 4 k-chunks per mask quarter

    with (
        tc.tile_pool(name="const", bufs=1) as constp,
        tc.tile_pool(name="qkslab", bufs=2) as qkp,
        tc.tile_pool(name="scratch", bufs=2, space="DRAM") as scrp,
        tc.tile_pool(name="vst", bufs=4) as vp,
        tc.tile_pool(name="maskp", bufs=8) as maskp,
        tc.tile_pool(name="ptp", bufs=6) as ptp,
        tc.tile_pool(name="osbp", bufs=10) as osbp,
        tc.tile_pool(name="ofp", bufs=4) as ofp,
        tc.tile_pool(name="rcp", bufs=4) as rcp,
        tc.tile_pool(name="ps_s", bufs=2, space="PSUM") as ps_s,
        tc.tile_pool(name="ps_po", bufs=2, space="PSUM") as ps_po,
        tc.tile_pool(name="ps_e", bufs=2, space="PSUM") as ps_e,
    ):
        identF = constp.tile([P, P], dt.float32)
        make_identity(nc, identF)

        for pr in range(n_pairs):
            heads = (2 * pr, 2 * pr + 1)

            # ---- mask quarter slabs: [p, qc, kl, j] bf16 (u8 cast in DMA) ----
            mslabs = [[None] * n_quart for _ in range(2)]
            for qt in range(n_quart):
                for hi, h in enumerate(heads):
                    ms = maskp.tile([P, n_qc * KLQ * QCH], dt.bfloat16, tag="ms")
                    src = m_ap[h, qt * KLQ * P : (qt + 1) * KLQ * P, :].rearrange(
                        "(kl p) (qc j) -> p qc kl j", p=P, j=QCH
                    )
                    dst = ms[:].rearrange("p (qc kl j) -> p qc kl j", qc=n_qc, kl=KLQ)
                    nc.gpsimd.dma_start(dst, src)
                    mslabs[hi][qt] = ms

            # ---- Q/K: cast+interleave to DRAM scratch, xbar-transpose to slab ----
            slabs = {}
            for name, src_ap in (("q", q_ap), ("k", k_ap)):
                scr = scrp.tile([S, P], dt.bfloat16, tag=f"scr_{name}")
                for hi, h in enumerate(heads):
                    nc.gpsimd.dma_start(scr[:, hi * D : (hi + 1) * D], src_ap[h])
                slab = qkp.tile([P, S], dt.bfloat16, tag=f"{name}t2")
                nc.sync.dma_start(slab[:], scr[:], transpose=True)
                slabs[name] = slab
            QT2, KT2 = slabs["q"], slabs["k"]

            # ---- V: [128, 16*65] chunk-major with ones column ----
            vst = [None, None]
            for hi, h in enumerate(heads):
                vt = vp.tile([P, n_kch * (D + 1)], dt.bfloat16, tag="vst")
                vt3 = vt[:].rearrange("p (t c) -> p t c", c=D + 1)
                nc.gpsimd.dma_start(
                    vt3[:, :, 0:D], v_ap[h].rearrange("(t p) d -> p t d", p=P)
                )
                nc.vector.memset(vt3[:, :, D : D + 1], 1.0)
                vst[hi] = vt

            # ---- main loop: halves of k, then q-chunks, PSUM-accumulated AV ----
            osb = {}
            for half in range(2):
                for qc in range(n_qc):
                    q0 = qc * QCH
                    po = [
                        ps_po.tile(
                            [D + 1, QCH], dt.float32, tag="po", name=f"po{hi_}"
                        )
                        for hi_ in range(2)
                    ]
                    for kg in range(NKH // 2):  # pairs of k-chunks
                        for hi in range(2):
                            st = ps_s.tile([P, 2 * QCH], dt.float32, tag="st")
                            for h2 in range(2):
                                ki = half * NKH + 2 * kg + h2
                                nc.tensor.matmul(
                                    st[:, h2 * QCH : (h2 + 1) * QCH],
                                    KT2[hi * D : (hi + 1) * D, ki * KCH : (ki + 1) * KCH],
                                    QT2[hi * D : (hi + 1) * D, q0 : q0 + QCH],
                                    start=True,
                                    stop=True,
                                )
                            pt = ptp.tile([P, 2 * QCH], dt.bfloat16, tag="pt")
                            nc.scalar.activation(
                                pt[:],
                                st[:],
                                mybir.ActivationFunctionType.Exp,
                                scale=SCALE,
                            )
                            # mask multiply: one contiguous [128, 1024] slice
                            kiq = half * NKH + 2 * kg  # first chunk of the pair
                            qt, klq = divmod(kiq, KLQ)
                            ms = mslabs[hi][qt]
                            off = (qc * KLQ + klq) * QCH
                            nc.vector.tensor_mul(
                                pt[:], pt[:], ms[:, off : off + 2 * QCH]
                            )
                            for h2 in range(2):
                                ki = half * NKH + 2 * kg + h2
                                nc.tensor.matmul(
                                    po[hi][:],
                                    vst[hi][:, ki * (D + 1) : (ki + 1) * (D + 1)],
                                    pt[:, h2 * QCH : (h2 + 1) * QCH],
                                    start=(kg == 0 and h2 == 0),
                                    stop=(kg == NKH // 2 - 1 and h2 == 1),
                                    skip_group_check=True,
                                )
                    # combine halves in SBUF (one copy + one add per (qc, hi))
                    for hi in range(2):
                        if half == 0:
                            ot_acc = osbp.tile([D + 1, QCH], dt.float32, tag="osb")
                            nc.vector.tensor_copy(ot_acc[:], po[hi][:])
                            osb[(qc, hi)] = ot_acc
                        else:
                            nc.vector.tensor_add(
                                osb[(qc, hi)][:], osb[(qc, hi)][:], po[hi][:]
                            )

                    if half == 0:
                        continue
                    # ---- epilogue: transpose, normalize, store ----
                    for hi, h in enumerate(heads):
                        acc = osb[(qc, hi)]
                        pst = ps_e.tile([P, 4 * (D + 1)], dt.float32, tag="pst")
                        for ot in range(4):
                            nc.tensor.transpose(
                                pst[:, ot * (D + 1) : (ot + 1) * (D + 1)],
                                acc[:, ot * P : (ot + 1) * P],
                                identF[0 : D + 1, 0 : D + 1],
                            )
                        rc = rcp.tile([P, 4], dt.float32, tag="rc")
                        nc.vector.reciprocal(
                            rc[:].rearrange("p (ot c) -> p ot c", c=1),
                            pst[:].rearrange("p (ot c) -> p ot c", c=D + 1)[
                                :, :, D : D + 1
                            ],
                        )
                        of = ofp.tile([P, 4 * D], dt.float32, tag="of")
                        for ot in range(4):
                            nc.vector.tensor_scalar_mul(
                                of[:, ot * D : (ot + 1) * D],
                                pst[:, ot * (D + 1) : ot * (D + 1) + D],
                                rc[:, ot : ot + 1],
                            )
                        nc.sync.dma_start(
                            o_ap[h, q0 : q0 + QCH, :].rearrange(
                                "(ot p) d -> p ot d", p=P
                            ),
                            of[:].rearrange("p (ot d) -> p ot d", d=D),
                        )


def build_program(H=H_PER_CORE, S=S_FULL, **flags):
    nc = bacc.Bacc()
    q = nc.dram_tensor("q", [H, S, D], mybir.dt.float32, kind="ExternalInput")
    k = nc.dram_tensor("k", [H, S, D], mybir.dt.float32, kind="ExternalInput")
    v = nc.dram_tensor("v", [H, S, D], mybir.dt.float32, kind="ExternalInput")
    m = nc.dram_tensor("m", [H, S, S], mybir.dt.uint8, kind="ExternalInput")
    o = nc.dram_tensor("o", [H, S, D], mybir.dt.float32, kind="ExternalOutput")
    with tile.TileContext(nc) as tc:
        build_attention(tc, o.ap(), q.ap(), k.ap(), v.ap(), m.ap(), H=H, S=S, **flags)
    nc.compile()
    return nc


_CACHE = {}
LAST_RESULTS = None


def kernel(queries, keys, values, mask):
    global LAST_RESULTS
    if "nc" not in _CACHE:
        _CACHE["nc"] = build_program()
    nc = _CACHE["nc"]

    queries = np.ascontiguousarray(queries, dtype=np.float32)
    keys = np.ascontiguousarray(keys, dtype=np.float32)
    values = np.ascontiguousarray(values, dtype=np.float32)
    # ship the KEEP mask (1 = keep) transposed ([BH, k, q]), u8
    keep_u8 = np.ascontiguousarray(
        (~np.asarray(mask)).transpose(0, 2, 1)
    ).view(np.uint8)

    in_maps = []
    for c in range(N_CORES):
        sl = slice(c * H_PER_CORE, (c + 1) * H_PER_CORE)
        in_maps.append(
            {
                "q": queries[sl],
                "k": keys[sl],
                "v": values[sl],
                "m": keep_u8[sl],
            }
        )

    trace = bool(int(os.environ.get("ATTN_TRACE", "0")))
    res = run_bass_kernel_spmd(
        nc, in_maps, core_ids=list(range(N_CORES)), trace=trace
    )
    LAST_RESULTS = res
    return np.concatenate([r["o"] for r in res.results], axis=0)


# revision 6
# speedup vs baseline: 1.9957x; 1.1256x over previous
"""Masked multi-head attention kernel for Trainium2 (Bass/Tile), 8-core SPMD.

Problem: BH=64 heads of S=2048, D=64 attention with a dense bool mask,
scale = 1/sqrt(1024).  Sharded 8 heads per NeuronCore (no cross-core comm).

Per-core dataflow (heads processed in pairs; ACT-exp is the design bottleneck):
  - Q,K: SWDGE cast-DMA f32->bf16 HBM->HBM into a pair-interleaved scratch
    [S, 128] (head A in cols 0:64, head B in 64:128), then ONE HWDGE xbar
    transpose-DMA per (pair, tensor) builds the [d2, S] slab directly in SBUF.
    No PE transposes, no DVE evacuation for the slabs.
  - V: SWDGE cast-DMA into [128, 16*65] chunk-major tiles; col 64 of each
    65-wide group memset to 1.0 once (gives softmax denominators in AV).
  - mask: host sends KEEP mask (1=keep) transposed [H, S_k, S_q] u8; SWDGE
    cast-DMA u8->bf16 into per-quarter slabs laid out [p, qc, kl, j] so the
    multiply operand for each (qc, k-pair) is one contiguous [128, 1024].
  - S^T = K Q^T row-paired on the PE (head A rows 0:64, head B rows 64:128,
    alternating -> LDWEIGHTS hides).  exp on ACT (scale=1/32) PSUM->SBUF bf16.
  - mask applied POST-exp as one DVE tensor_mul [128,1024] (bf16 2x mode);
    exact zeros for masked entries.
  - AV: stationary [V|1] (M=65) streams masked P^T, accumulating over the 8
    k-chunks of a half directly in PSUM; halves combined with one DVE add.
  - Epilogue per (qc, head): 4 PE transposes into one packed PSUM tile
    [128, 4*65], one strided reciprocal, 4 per-partition scales, one store.
"""

import os
import sys

sys.path.insert(0, "/opt/trn_rl_repo")

import numpy as np

import concourse.bass as bass
import concourse.mybir as mybir
import concourse.tile as tile
from concourse import bacc
from concourse.bass_utils import run_bass_kernel_spmd
from concourse.masks import make_identity

N_CORES = 8
BH, S_FULL, D = 64, 2048, 64
H_PER_CORE = BH // N_CORES  # 8
P = 128  # SBUF/PSUM partitions
KCH = 128  # k-chunk rows
QCH = 512  # q-chunk cols
SCALE = 1.0 / 32.0  # 1/sqrt(1024) per the module spec


def build_attention(tc, o_ap, q_ap, k_ap, v_ap, m_ap, H, S):
    nc = tc.nc
    dt = mybir.dt
    n_pairs = H // 2
    n_kch = S // KCH  # 16 k-chunks per head
    n_qc = S // QCH  # 4 q-chunks
    NKH = n_kch // 2  # 8 k-chunks per half
    n_quart = 4  # mask DMA granularity: quarter of the k range
    KLQ = n_kch // n_quart  # 4 k-chunks per mask quarter

    with (
        tc.tile_pool(name="const", bufs=1) as constp,
        tc.tile_pool(name="qkslab", bufs=2) as qkp,
        tc.tile_pool(name="scratch", bufs=2, space="DRAM") as scrp,
        tc.tile_pool(name="vst", bufs=4) as vp,
        tc.tile_pool(name="maskp", bufs=8) as maskp,
        tc.tile_pool(name="ptp", bufs=8) as ptp,
        tc.tile_pool(name="osbp", bufs=10) as osbp,
        tc.tile_pool(name="ofp", bufs=4) as ofp,
        tc.tile_pool(name="rcp", bufs=4) as rcp,
        tc.tile_pool(name="ps_s", bufs=2, space="PSUM") as ps_s,
        tc.tile_pool(name="ps_po", bufs=2, space="PSUM") as ps_po,
        tc.tile_pool(name="ps_e", bufs=2, space="PSUM") as ps_e,
    ):
        identF = constp.tile([P, P], dt.float32)
        make_identity(nc, identF)
        # PE warmup: keep the PE busy through the initial DMA wait so the HAM
        # clock gate reaches K=8/8 before (and holds through) the first real
        # matmul.  ~30 back-to-back N=512 matmuls on a zero tile.
        wsrc = constp.tile([P, QCH], dt.bfloat16)
        nc.vector.memset(wsrc[:], 0.0)
        wps = ps_s.tile([P, 2 * QCH], dt.float32, tag="st")
        for _ in range(30):
            nc.tensor.matmul(
                wps[:, 0:QCH], wsrc[:, 0:P], wsrc[:], start=True, stop=True
            )

        for pr in range(n_pairs):
            heads = (2 * pr, 2 * pr + 1)

            # ---- Q/K: cast+interleave to DRAM scratch, xbar-transpose to slab ----
            slabs = {}
            for name, src_ap in (("q", q_ap), ("k", k_ap)):
                scr = scrp.tile([S, P], dt.bfloat16, tag=f"scr_{name}")
                for hi, h in enumerate(heads):
                    nc.gpsimd.dma_start(scr[:, hi * D : (hi + 1) * D], src_ap[h])
                slab = qkp.tile([P, S], dt.bfloat16, tag=f"{name}t2")
                nc.sync.dma_start(slab[:], scr[:], transpose=True)
                slabs[name] = slab
            QT2, KT2 = slabs["q"], slabs["k"]

            # ---- V: [128, 16*65] chunk-major with ones column ----
            vst = [None, None]
            for hi, h in enumerate(heads):
                vt = vp.tile([P, n_kch * (D + 1)], dt.bfloat16, tag="vst")
                vt3 = vt[:].rearrange("p (t c) -> p t c", c=D + 1)
                nc.gpsimd.dma_start(
                    vt3[:, :, 0:D], v_ap[h].rearrange("(t p) d -> p t d", p=P)
                )
                nc.vector.memset(vt3[:, :, D : D + 1], 1.0)
                vst[hi] = vt

            # ---- mask quarter slabs: [p, qc, kl, j] bf16 (u8 cast in DMA) ----
            # Issued AFTER the Q/K/V loads: SWDGE transfers drain in FIFO
            # order, and the slabs must not queue behind 16 MB of mask data.
            mslabs = [[None] * n_quart for _ in range(2)]
            for qt in range(n_quart):
                for hi, h in enumerate(heads):
                    ms = maskp.tile([P, n_qc * KLQ * QCH], dt.bfloat16, tag="ms")
                    src = m_ap[h, qt * KLQ * P : (qt + 1) * KLQ * P, :].rearrange(
                        "(kl p) (qc j) -> p qc kl j", p=P, j=QCH
                    )
                    dst = ms[:].rearrange("p (qc kl j) -> p qc kl j", qc=n_qc, kl=KLQ)
                    nc.gpsimd.dma_start(dst, src)
                    mslabs[hi][qt] = ms

            # ---- main loop: halves of k, then q-chunks, PSUM-accumulated AV ----
            osb = {}
            for half in range(2):
                for qc in range(n_qc):
                    q0 = qc * QCH
                    po = [
                        ps_po.tile(
                            [D + 1, QCH], dt.float32, tag="po", name=f"po{hi_}"
                        )
                        for hi_ in range(2)
                    ]
                    for kg in range(NKH // 2):  # pairs of k-chunks
                        for hi in range(2):
                            st = ps_s.tile([P, 2 * QCH], dt.float32, tag="st")
                            for h2 in range(2):
                                ki = half * NKH + 2 * kg + h2
                                nc.tensor.matmul(
                                    st[:, h2 * QCH : (h2 + 1) * QCH],
                                    KT2[hi * D : (hi + 1) * D, ki * KCH : (ki + 1) * KCH],
                                    QT2[hi * D : (hi + 1) * D, q0 : q0 + QCH],
                                    start=True,
                                    stop=True,
                                )
                            pt = ptp.tile([P, 2 * QCH], dt.bfloat16, tag="pt")
                            nc.scalar.activation(
                                pt[:],
                                st[:],
                                mybir.ActivationFunctionType.Exp,
                                scale=SCALE,
                            )
                            # mask multiply: one contiguous [128, 1024] slice
                            kiq = half * NKH + 2 * kg  # first chunk of the pair
                            qt, klq = divmod(kiq, KLQ)
                            ms = mslabs[hi][qt]
                            off = (qc * KLQ + klq) * QCH
                            nc.vector.tensor_mul(
                                pt[:], pt[:], ms[:, off : off + 2 * QCH]
                            )
                            for h2 in range(2):
                                ki = half * NKH + 2 * kg + h2
                                nc.tensor.matmul(
                                    po[hi][:],
                                    vst[hi][:, ki * (D + 1) : (ki + 1) * (D + 1)],
                                    pt[:, h2 * QCH : (h2 + 1) * QCH],
                                    start=(kg == 0 and h2 == 0),
                                    stop=(kg == NKH // 2 - 1 and h2 == 1),
                                    skip_group_check=True,
                                )
                    # combine halves in SBUF (one copy + one add per (qc, hi))
                    for hi in range(2):
                        if half == 0:
                            ot_acc = osbp.tile([D + 1, QCH], dt.float32, tag="osb")
                            nc.vector.tensor_copy(ot_acc[:], po[hi][:])
                            osb[(qc, hi)] = ot_acc
                        else:
                            nc.vector.tensor_add(
                                osb[(qc, hi)][:], osb[(qc, hi)][:], po[hi][:]
                            )

                    if half == 0:
                        continue
                    # ---- epilogue: transpose, normalize, store ----
                    for hi, h in enumerate(heads):
                        acc = osb[(qc, hi)]
                        pst = ps_e.tile([P, 4 * (D + 1)], dt.float32, tag="pst")
                        for ot in range(4):
                            nc.tensor.transpose(
                                pst[:, ot * (D + 1) : (ot + 1) * (D + 1)],
                                acc[:, ot * P : (ot + 1) * P],
                                identF[0 : D + 1, 0 : D + 1],
                            )
                        rc = rcp.tile([P, 4], dt.float32, tag="rc")
                        nc.vector.reciprocal(
                            rc[:].rearrange("p (ot c) -> p ot c", c=1),
                            pst[:].rearrange("p (ot c) -> p ot c", c=D + 1)[
                                :, :, D : D + 1
                            ],
                        )
                        of = ofp.tile([P, 4 * D], dt.float32, tag="of")
                        for ot in range(4):
                            nc.vector.tensor_scalar_mul(
                                of[:, ot * D : (ot + 1) * D],
                                pst[:, ot * (D + 1) : ot * (D + 1) + D],
                                rc[:, ot : ot + 1],
                            )
                        nc.sync.dma_start(
                            o_ap[h, q0 : q0 + QCH, :].rearrange(
                                "(ot p) d -> p ot d", p=P
                            ),
                            of[:].rearrange("p (ot d) -> p ot d", d=D),
                        )


def build_program(H=H_PER_CORE, S=S_FULL, **flags):
    nc = bacc.Bacc()
    q = nc.dram_tensor("q", [H, S, D], mybir.dt.float32, kind="ExternalInput")
    k = nc.dram_tensor("k", [H, S, D], mybir.dt.float32, kind="ExternalInput")
    v = nc.dram_tensor("v", [H, S, D], mybir.dt.float32, kind="ExternalInput")
    m = nc.dram_tensor("m", [H, S, S], mybir.dt.uint8, kind="ExternalInput")
    o = nc.dram_tensor("o", [H, S, D], mybir.dt.float32, kind="ExternalOutput")
    with tile.TileContext(nc) as tc:
        build_attention(tc, o.ap(), q.ap(), k.ap(), v.ap(), m.ap(), H=H, S=S, **flags)
    nc.compile()
    return nc


_CACHE = {}
LAST_RESULTS = None


def kernel(queries, keys, values, mask):
    global LAST_RESULTS
    if "nc" not in _CACHE:
        _CACHE["nc"] = build_program()
    nc = _CACHE["nc"]

    queries = np.ascontiguousarray(queries, dtype=np.float32)
    keys = np.ascontiguousarray(keys, dtype=np.float32)
    values = np.ascontiguousarray(values, dtype=np.float32)
    # ship the KEEP mask (1 = keep) transposed ([BH, k, q]), u8
    keep_u8 = np.ascontiguousarray(
        (~np.asarray(mask)).transpose(0, 2, 1)
    ).view(np.uint8)

    in_maps = []
    for c in range(N_CORES):
        sl = slice(c * H_PER_CORE, (c + 1) * H_PER_CORE)
        in_maps.append(
            {
                "q": queries[sl],
                "k": keys[sl],
                "v": values[sl],
                "m": keep_u8[sl],
            }
        )

    trace = bool(int(os.environ.get("ATTN_TRACE", "0")))
    res = run_bass_kernel_spmd(
        nc, in_maps, core_ids=list(range(N_CORES)), trace=trace
    )
    LAST_RESULTS = res
    return np.concatenate([r["o"] for r in res.results], axis=0)
